# revision 1
# baseline (speedup 1.0000x reference)
"""Trainium2 Bass kernel for LoRA attention prefill (B=4, S=1024, D=4096, H=32).

Sharding: tensor-parallel over heads. Each of the 8 cores computes 4 heads
(512 of the 4096 q/k/v features, column-shard of wq/wk/wv + LoRA B) and a
row-shard of wo, producing a full-shape [T, D] partial output; partials are
summed on the host.

Device layout choices:
  - All matmuls in bf16 with fp32 PSUM accumulation.
  - Activations kept feature-on-partition ("transposed"): xT [D, T] feeds
    Q/K projections directly as PE operands; V is computed token-on-partition
    so it can serve as the PV-matmul stationary operand without transposes.
  - RoPE handled by host-permuting wq/wk rows so each head's real/imag halves
    land in lane-aligned 64-partition blocks of separate psum tiles; scores
    then contract each head with two K=64 matmuls.
  - Attention in "scoresT" layout (keys on partitions): softmax denominator
    via a ones-column matmul on PE, probs feed the PV matmul untransposed,
    normalization applied to the attention output via a PE broadcast of the
    reciprocal sums.
"""
import sys
from contextlib import ExitStack

sys.path.insert(0, "/opt/trn_rl_repo")

import numpy as np
import ml_dtypes

import concourse.bass as bass
import concourse.mybir as mybir
import concourse.tile as tile
from concourse import bacc
from concourse.bass_utils import run_bass_kernel_spmd
from concourse.tile import TileContext

B, S, D = 4, 1024, 4096
H, HD = 32, 128
R = 16
LORA_SCALE = 2.0
N_CORES = 8
HPC = H // N_CORES            # heads per core
FPC = HPC * HD                # features per core = 512
T = B * S                     # 4096 tokens
TT = 256                      # stage-A T-tile (tokens)
NTT = S // TT                 # T-tiles per batch = 4
SCALE = float(1.0 / np.sqrt(HD))
BF = mybir.dt.bfloat16
F32 = mybir.dt.float32


def _bf(a):
    return np.ascontiguousarray(np.asarray(a, np.float32).astype(ml_dtypes.bfloat16))


def _core_perm(c):
    hs = [HPC * c + i for i in range(HPC)]
    ev = np.arange(0, HD, 2)
    od = np.arange(1, HD, 2)
    out = []
    for pair in (0, 1):
        h0, h1 = hs[2 * pair], hs[2 * pair + 1]
        out.append(h0 * HD + ev)
        out.append(h1 * HD + ev)
        out.append(h0 * HD + od)
        out.append(h1 * HD + od)
    return np.concatenate(out)


def _check_causal(mask):
    iu = np.triu_indices(S, k=1)
    il = np.tril_indices(S, k=0)
    return bool((mask[iu] <= -1e8).all() and (mask[il] == 0).all())


def _host_prep(x, wq_w, wq_a, wq_b, wk_w, wv_w, wv_a, wv_b, wo_w,
               freqs_cos, freqs_sin, mask):
    x2 = np.asarray(x, np.float32).reshape(T, D)
    xT = _bf(x2.T)
    waT = np.zeros((D, 48), np.float32)
    waT[:, 0:16] = np.asarray(wq_a, np.float32).T
    waT[:, 32:48] = np.asarray(wv_a, np.float32).T
    waT = _bf(waT)

    cosT = np.asarray(freqs_cos, np.float32).T
    sinT = np.asarray(freqs_sin, np.float32).T
    cc = np.ascontiguousarray(np.tile(cosT, (2, B)).astype(np.float32))
    ss = np.ascontiguousarray(np.tile(sinT, (2, B)).astype(np.float32))

    mask = np.asarray(mask, np.float32)
    causal = _check_causal(mask)
    mT = mask.T * np.float32(np.sqrt(HD))
    if causal:
        # diag-band blocks are identical for both query halves; verify
        maskp = np.zeros((4, 128, 512), np.float32)
        for j in range(4):
            maskp[j] = mT[j * 128:(j + 1) * 128, 0:512]
            if not np.array_equal(
                    maskp[j], mT[(4 + j) * 128:(5 + j) * 128, 512:1024]):
                causal = False
                break
    if not causal:
        maskp = np.zeros((8, 128, 2, 512), np.float32)
        for qh in range(2):
            for j in range(8):
                maskp[j, :, qh, :] = mT[j * 128:(j + 1) * 128,
                                        qh * 512:(qh + 1) * 512]

    shared = dict(xT=xT, waT=waT, cc=cc, ss=ss, maskp=maskp)
    cores = []
    for c in range(N_CORES):
        perm = _core_perm(c)
        sl = slice(c * FPC, (c + 1) * FPC)
        cores.append(dict(
            wqT=_bf(np.asarray(wq_w, np.float32)[perm, :].T),
            wkT=_bf(np.asarray(wk_w, np.float32)[perm, :].T),
            wvT=_bf(np.asarray(wv_w, np.float32)[sl, :].T),
            wqbT=_bf((np.asarray(wq_b, np.float32)[perm, :] * LORA_SCALE).T),
            wvbT=_bf((np.asarray(wv_b, np.float32)[sl, :] * LORA_SCALE).T),
            woT=_bf(np.asarray(wo_w, np.float32)[:, sl].T),
        ))
    return shared, cores, causal


def _build_program(causal):
    nc = bacc.Bacc("TRN2", num_devices=N_CORES)
    dt = mybir.dt
    nkb = 4 if causal else 8

    xT = nc.dram_tensor("xT", [D, T], BF, kind="ExternalInput").ap()
    wqT = nc.dram_tensor("wqT", [D, FPC], BF, kind="ExternalInput").ap()
    wkT = nc.dram_tensor("wkT", [D, FPC], BF, kind="ExternalInput").ap()
    wvT = nc.dram_tensor("wvT", [D, FPC], BF, kind="ExternalInput").ap()
    waT = nc.dram_tensor("waT", [D, 48], BF, kind="ExternalInput").ap()
    wqbT = nc.dram_tensor("wqbT", [R, FPC], BF, kind="ExternalInput").ap()
    wvbT = nc.dram_tensor("wvbT", [R, FPC], BF, kind="ExternalInput").ap()
    woT = nc.dram_tensor("woT", [FPC, D], BF, kind="ExternalInput").ap()
    cc = nc.dram_tensor("cc", [128, T], F32, kind="ExternalInput").ap()
    ss = nc.dram_tensor("ss", [128, T], F32, kind="ExternalInput").ap()
    mshape = [4, 128, 512] if causal else [8, 128, 2, 512]
    maskp = nc.dram_tensor("maskp", mshape, F32, kind="ExternalInput").ap()
    y = nc.dram_tensor("y", [T, D], F32, kind="ExternalOutput").ap()

    with TileContext(nc) as tc, ExitStack() as ctx:
        wpool = ctx.enter_context(tc.tile_pool(name="wpool", bufs=1))
        xpool = ctx.enter_context(tc.tile_pool(name="xpool", bufs=2))
        ccp = ctx.enter_context(tc.tile_pool(name="ccp", bufs=1))
        qkvp = ctx.enter_context(tc.tile_pool(name="qkvp", bufs=1))
        xap = ctx.enter_context(tc.tile_pool(name="xap", bufs=2))
        expp = ctx.enter_context(tc.tile_pool(name="expp", bufs=3))
        otp = ctx.enter_context(tc.tile_pool(name="otp", bufs=1))
        outp = ctx.enter_context(tc.tile_pool(name="outp", bufs=3))
        tmpp = ctx.enter_context(tc.tile_pool(name="tmpp", bufs=6))
        stp = ctx.enter_context(tc.tile_pool(name="stp", bufs=4))
        sump = ctx.enter_context(tc.tile_pool(name="sump", bufs=1))
        wop = ctx.enter_context(tc.tile_pool(name="wop", bufs=2))
        psa = ctx.enter_context(tc.tile_pool(name="psac", bufs=3, space="PSUM"))
        psc = psa
        psb = ctx.enter_context(tc.tile_pool(name="psb", bufs=5, space="PSUM"))

        # resident weights
        wq_sb = wpool.tile([128, 32, FPC], BF, tag="wq")
        nc.sync.dma_start(wq_sb[:], wqT.rearrange("(o p) f -> p o f", p=128))
        wk_sb = wpool.tile([128, 32, FPC], BF, tag="wk")
        nc.sync.dma_start(wk_sb[:], wkT.rearrange("(o p) f -> p o f", p=128))
        wv_sb = wpool.tile([128, 32, FPC], BF, tag="wv")
        nc.sync.dma_start(wv_sb[:], wvT.rearrange("(o p) f -> p o f", p=128))
        wa_sb = wpool.tile([128, 32, 48], BF, tag="wa")
        nc.sync.dma_start(wa_sb[:], waT.rearrange("(o p) f -> p o f", p=128))
        wqb_sb = wpool.tile([R, FPC], BF, tag="wqb")
        nc.sync.dma_start(wqb_sb[:], wqbT[:])
        # parked at partitions 32-47 so the V-lora matmul's lhsT (xa rows
        # 32-47) and rhs share a base partition, as the PE requires
        wvb_sb = wpool.tile([48, FPC], BF, tag="wvb")
        nc.sync.dma_start(wvb_sb[32:48, :], wvbT[:])
        if causal:
            mask_sb = wpool.tile([128, 4, 512], F32, tag="mask")
            nc.sync.dma_start(mask_sb[:], maskp.rearrange("j p n -> p j n"))
        else:
            mask_sb = wpool.tile([128, 8, 2, 512], F32, tag="mask")
            nc.sync.dma_start(mask_sb[:],
                              maskp.rearrange("j p q n -> p j q n"))
        ones_col = wpool.tile([128, 1], BF, tag="onec")
        nc.gpsimd.memset(ones_col[:], 1.0)
        ones_row = wpool.tile([1, 128], F32, tag="oner")
        nc.gpsimd.memset(ones_row[:], 1.0)

        for b in range(B):
            Q_sb = qkvp.tile([128, 4, S], BF, tag="Qsb")
            K_sb = qkvp.tile([128, 4, S], BF, tag="Ksb")
            V_sb = qkvp.tile([128, 8, FPC], BF, tag="Vsb")
            OT_sb = otp.tile([128, 4, S], BF, tag="OT")

            def attn_half(qh):
                q0 = qh * 512
                kbs = list(range(0, qh * 4 + 4)) if causal else list(range(8))
                last = len(kbs) - 1
                for l in range(HPC):
                    ps_ot = psb.tile([128, 512], F32, tag="psb")
                    ps_sum = psb.tile([128, 512], F32, tag="psb")
                    for j, kb in enumerate(kbs):
                        k0 = kb * 128
                        ps_sc = psb.tile([128, 512], F32, tag="psb")
                        nc.tensor.matmul(
                            ps_sc[:], K_sb[:, l, k0:k0 + 128],
                            Q_sb[:, l, q0:q0 + 512], start=True, stop=True)
                        if causal:
                            if kb >= qh * 4:
                                nc.vector.tensor_add(
                                    ps_sc[:], ps_sc[:],
                                    mask_sb[:, kb - qh * 4, :])
                        else:
                            nc.vector.tensor_add(
                                ps_sc[:], ps_sc[:], mask_sb[:, kb, qh, :])
                        e_sb = expp.tile([128, 512], BF, tag="e")
                        nc.scalar.activation(
                            e_sb[:], ps_sc[:],
                            mybir.ActivationFunctionType.Exp, scale=SCALE)
                        nc.tensor.matmul(ps_sum[0:1, :], ones_col[:], e_sb[:],
                                         start=(j == 0), stop=(j == last))
                        nc.tensor.matmul(
                            ps_ot[:], V_sb[:, kb, l * 128:(l + 1) * 128],
                            e_sb[:], start=(j == 0), stop=(j == last))
                    # normalization: keep the slow reciprocal off PSUM so the
                    # next head's matmuls aren't starved of banks
                    sum_sb = sump.tile([1, 512], F32, tag="sum")
                    nc.scalar.copy(sum_sb[:], ps_sum[0:1, :])
                    rec1_sb = sump.tile([1, 512], F32, tag="rec1")
                    nc.vector.reciprocal(rec1_sb[:], sum_sb[:])
                    ps_bc = psb.tile([128, 512], F32, tag="psb")
                    nc.tensor.matmul(ps_bc[:], ones_row[:], rec1_sb[:],
                                     start=True, stop=True)
                    rec_sb = outp.tile([128, 512], F32, tag="o")
                    nc.vector.tensor_copy(rec_sb[:], ps_bc[:])
                    nc.vector.tensor_mul(OT_sb[:, l, q0:q0 + 512], ps_ot[:],
                                         rec_sb[:])

            # ---- stage A: projections + RoPE (attention qh=0 interleaved) --
            for tt in range(NTT):
                t0 = b * S + tt * TT
                x_sb = xpool.tile([128, 32, TT], BF, tag="x")
                nc.sync.dma_start(
                    x_sb[:],
                    xT.rearrange("(o p) t -> p o t", p=128)[:, :, t0:t0 + TT])
                cc_sb = ccp.tile([128, TT], F32, tag="cc")
                nc.sync.dma_start(cc_sb[:], cc[:, t0:t0 + TT])
                ss_sb = ccp.tile([128, TT], F32, tag="ss")
                nc.sync.dma_start(ss_sb[:], ss[:, t0:t0 + TT])

                # lora A: xa[48, TT]
                ps_xa = psa.tile([128, 512], F32, tag="psa")
                for d in range(32):
                    nc.tensor.matmul(ps_xa[0:48, 0:TT], wa_sb[:, d, :],
                                     x_sb[:, d, :], start=(d == 0),
                                     stop=(d == 31))
                xa_sb = xap.tile([48, TT], BF, tag="xa")
                nc.scalar.copy(xa_sb[:], ps_xa[0:48, 0:TT])

                # Q and K, RoPE'd into Q_sb/K_sb
                for dst_sb, w_sb, lora in ((Q_sb, wq_sb, True),
                                           (K_sb, wk_sb, False)):
                    for pair in range(2):
                        ps_pair = []
                        for ri in range(2):
                            f0 = pair * 256 + ri * 128
                            ps = psa.tile([128, 512], F32, tag="psa")
                            for d in range(32):
                                nc.tensor.matmul(
                                    ps[:, 0:TT], w_sb[:, d, f0:f0 + 128],
                                    x_sb[:, d, :], start=(d == 0),
                                    stop=(d == 31 and not lora))
                            if lora:
                                nc.tensor.matmul(
                                    ps[:, 0:TT], wqb_sb[:, f0:f0 + 128],
                                    xa_sb[0:16, :], start=False, stop=True)
                            ps_pair.append(ps)
                        ps_r, ps_i = ps_pair
                        toff = tt * TT
                        t1 = tmpp.tile([128, TT], F32, tag="t")
                        nc.vector.tensor_mul(t1[:], ps_r[:, 0:TT], cc_sb[:])
                        t2 = tmpp.tile([128, TT], F32, tag="t")
                        nc.vector.tensor_mul(t2[:], ps_i[:, 0:TT], ss_sb[:])
                        st_r = stp.tile([128, TT], BF, tag="st")
                        nc.vector.tensor_tensor(
                            st_r[:], t1[:], t2[:], mybir.AluOpType.subtract)
                        t3 = tmpp.tile([128, TT], F32, tag="t")
                        nc.vector.tensor_mul(t3[:], ps_r[:, 0:TT], ss_sb[:])
                        t4 = tmpp.tile([128, TT], F32, tag="t")
                        nc.vector.tensor_mul(t4[:], ps_i[:, 0:TT], cc_sb[:])
                        st_i = stp.tile([128, TT], BF, tag="st")
                        nc.vector.tensor_tensor(
                            st_i[:], t3[:], t4[:], mybir.AluOpType.add)
                        # shuffle into head-contiguous blocks: head h of this
                        # pair = [r half; i half] on partitions [0:64|64:128]
                        for hh in range(2):
                            h_loc = 2 * pair + hh
                            nc.sync.dma_start(
                                dst_sb[0:64, h_loc, toff:toff + TT],
                                st_r[hh * 64:(hh + 1) * 64, :])
                            nc.sync.dma_start(
                                dst_sb[64:128, h_loc, toff:toff + TT],
                                st_i[hh * 64:(hh + 1) * 64, :])

                # V natural: per 128-token block
                for v in range(TT // 128):
                    tb = tt * (TT // 128) + v
                    ps_v = psa.tile([128, 512], F32, tag="psa")
                    for d in range(32):
                        nc.tensor.matmul(
                            ps_v[:], x_sb[:, d, v * 128:(v + 1) * 128],
                            wv_sb[:, d, :], start=(d == 0), stop=False)
                    nc.tensor.matmul(
                        ps_v[:], xa_sb[32:48, v * 128:(v + 1) * 128],
                        wvb_sb[32:48, :], start=False, stop=True)
                    nc.scalar.copy(V_sb[:, tb, :], ps_v[:])

                if tt == 1:
                    attn_half(0)
            attn_half(1)

            # ---- stage C: wo ----
            for nt in range(8):
                wo_sb = wop.tile([128, 4, 512], BF, tag="wo")
                nc.sync.dma_start(
                    wo_sb[:],
                    woT.rearrange("(o p) n -> p o n",
                                  p=128)[:, :, nt * 512:(nt + 1) * 512])
                for tb in range(8):
                    ps_o = psc.tile([128, 512], F32, tag="psa")
                    for k in range(4):
                        nc.tensor.matmul(
                            ps_o[:], OT_sb[:, k, tb * 128:(tb + 1) * 128],
                            wo_sb[:, k, :], start=(k == 0), stop=(k == 3))
                    o_sb = outp.tile([128, 512], F32, tag="o")
                    nc.scalar.copy(o_sb[:], ps_o[:])
                    nc.sync.dma_start(
                        y[b * S + tb * 128:b * S + (tb + 1) * 128,
                          nt * 512:(nt + 1) * 512], o_sb[:])

    nc.compile()
    return nc


_CACHE = {}


def _get_program(causal):
    if causal not in _CACHE:
        _CACHE[causal] = _build_program(causal)
    return _CACHE[causal]


def kernel(x, wq_w, wq_a, wq_b, wk_w, wv_w, wv_a, wv_b, wo_w,
           freqs_cos, freqs_sin, mask, start_pos=0, _trace=False):
    assert int(np.asarray(start_pos)) == 0
    shared, cores, causal = _host_prep(
        x, wq_w, wq_a, wq_b, wk_w, wv_w, wv_a, wv_b, wo_w,
        freqs_cos, freqs_sin, mask)
    nc = _get_program(causal)
    in_maps = []
    for c in range(N_CORES):
        m = dict(xT=shared["xT"], waT=shared["waT"], cc=shared["cc"],
                 ss=shared["ss"], maskp=shared["maskp"])
        m.update(cores[c])
        in_maps.append(m)
    res = run_bass_kernel_spmd(nc, in_maps, list(range(N_CORES)),
                               trace=_trace)
    kernel._last_results = res
    acc = np.zeros((T, D), np.float32)
    for c in range(N_CORES):
        acc += np.asarray(res.results[c]["y"], np.float32)
    out = acc.reshape(B, S, D)
    return out.astype(np.asarray(x).dtype, copy=False)



# revision 4
# speedup vs baseline: 1.2290x; 1.2290x over previous
"""Trainium2 Bass kernel for LoRA attention prefill (B=4, S=1024, D=4096, H=32).

Sharding: tensor-parallel over heads. Each of the 8 cores computes 4 heads
(512 of the 4096 q/k/v features, column-shard of wq/wk/wv) and a row-shard
of wo, producing a full-shape [T, D] partial output; partials are summed on
the host.

v2 design notes (vs the earlier baseline):
  - LoRA is folded into wq/wv on the host (exact algebra), removing all
    device-side LoRA matmuls.
  - Softmax denominators: exp tiles are accumulated on GpSimd into an SBUF
    acc; a single ones-matrix matmul per head broadcasts the denominator to
    all 128 partitions, and reciprocal_approx_fast (DVE) replaces the slow
    serial RECIPROCAL that used to stall the PE (and let HAM re-throttle).
  - Causal masking is multiplicative (0/1 bf16 after exp), and diagonal
    score blocks only compute the live query range (partial-width matmuls).
  - Attention is emitted as a 2-head software pipeline with PV matmuls
    lagging one key-block behind the score matmuls so the PE queue always
    has independent work while the Scalar engine computes exp.
"""
import sys
from contextlib import ExitStack

sys.path.insert(0, "/opt/trn_rl_repo")

import numpy as np
import ml_dtypes

import concourse.bass as bass
import concourse.mybir as mybir
import concourse.tile as tile
from concourse import bacc
from concourse.bass_utils import run_bass_kernel_spmd
from concourse.tile import TileContext

B, S, D = 4, 1024, 4096
H, HD = 32, 128
R = 16
LORA_SCALE = 2.0
N_CORES = 8
HPC = H // N_CORES            # heads per core
FPC = HPC * HD                # features per core = 512
T = B * S                     # 4096 tokens
TT = 256                      # stage-A T-tile (tokens)
NTT = S // TT                 # T-tiles per batch = 4
SCALE = float(1.0 / np.sqrt(HD))
BF = mybir.dt.bfloat16
F32 = mybir.dt.float32


def _bf(a):
    return np.ascontiguousarray(np.asarray(a, np.float32).astype(ml_dtypes.bfloat16))


def _core_perm(c):
    hs = [HPC * c + i for i in range(HPC)]
    ev = np.arange(0, HD, 2)
    od = np.arange(1, HD, 2)
    out = []
    for pair in (0, 1):
        h0, h1 = hs[2 * pair], hs[2 * pair + 1]
        out.append(h0 * HD + ev)
        out.append(h1 * HD + ev)
        out.append(h0 * HD + od)
        out.append(h1 * HD + od)
    return np.concatenate(out)


def _check_causal(mask):
    iu = np.triu_indices(S, k=1)
    il = np.tril_indices(S, k=0)
    return bool((mask[iu] <= -1e8).all() and (mask[il] == 0).all())


def _host_prep(x, wq_w, wq_a, wq_b, wk_w, wv_w, wv_a, wv_b, wo_w,
               freqs_cos, freqs_sin, mask):
    x2 = np.asarray(x, np.float32).reshape(T, D)
    xT = _bf(x2.T)

    # fold LoRA into the dense weights: y = x(W + s·B@A)^T exactly
    wq_eff = np.asarray(wq_w, np.float32) + LORA_SCALE * (
        np.asarray(wq_b, np.float32) @ np.asarray(wq_a, np.float32))
    wv_eff = np.asarray(wv_w, np.float32) + LORA_SCALE * (
        np.asarray(wv_b, np.float32) @ np.asarray(wv_a, np.float32))
    wk = np.asarray(wk_w, np.float32)

    cosT = np.asarray(freqs_cos, np.float32).T
    sinT = np.asarray(freqs_sin, np.float32).T
    cc = np.ascontiguousarray(np.tile(cosT, (2, B)).astype(np.float32))
    ss = np.ascontiguousarray(np.tile(sinT, (2, B)).astype(np.float32))

    mask = np.asarray(mask, np.float32)
    causal = _check_causal(mask)
    if causal:
        # 0/1 multiplicative triangle for the 128x128 diagonal strips
        tri = np.tril(np.ones((128, 128), np.float32)).T  # [k,q]: 1 if k<=q
        maskp = _bf(np.broadcast_to(tri, (4, 128, 128)))
    else:
        mT = mask.T * np.float32(np.sqrt(HD))
        maskp = np.zeros((8, 128, 2, 512), np.float32)
        for qh in range(2):
            for j in range(8):
                maskp[j, :, qh, :] = mT[j * 128:(j + 1) * 128,
                                        qh * 512:(qh + 1) * 512]

    shared = dict(xT=xT, cc=cc, ss=ss, maskp=maskp)
    cores = []
    for c in range(N_CORES):
        perm = _core_perm(c)
        sl = slice(c * FPC, (c + 1) * FPC)
        cores.append(dict(
            wqT=_bf(wq_eff[perm, :].T),
            wkT=_bf(wk[perm, :].T),
            wvT=_bf(wv_eff[sl, :].T),
            woT=_bf(np.asarray(wo_w, np.float32)[:, sl].T),
        ))
    return shared, cores, causal


def _build_program(causal):
    nc = bacc.Bacc("TRN2", num_devices=N_CORES)

    xT = nc.dram_tensor("xT", [D, T], BF, kind="ExternalInput").ap()
    wqT = nc.dram_tensor("wqT", [D, FPC], BF, kind="ExternalInput").ap()
    wkT = nc.dram_tensor("wkT", [D, FPC], BF, kind="ExternalInput").ap()
    wvT = nc.dram_tensor("wvT", [D, FPC], BF, kind="ExternalInput").ap()
    woT = nc.dram_tensor("woT", [FPC, D], BF, kind="ExternalInput").ap()
    cc = nc.dram_tensor("cc", [128, T], F32, kind="ExternalInput").ap()
    ss = nc.dram_tensor("ss", [128, T], F32, kind="ExternalInput").ap()
    if causal:
        maskp = nc.dram_tensor("maskp", [4, 128, 128], BF,
                               kind="ExternalInput").ap()
    else:
        maskp = nc.dram_tensor("maskp", [8, 128, 2, 512], F32,
                               kind="ExternalInput").ap()
    y = nc.dram_tensor("y", [T, D], F32, kind="ExternalOutput").ap()

    with TileContext(nc) as tc, ExitStack() as ctx:
        wpool = ctx.enter_context(tc.tile_pool(name="wpool", bufs=1))
        xpool = ctx.enter_context(tc.tile_pool(name="xpool", bufs=2))
        ccp = ctx.enter_context(tc.tile_pool(name="ccp", bufs=2))
        qkvp = ctx.enter_context(tc.tile_pool(name="qkvp", bufs=1))
        expp = ctx.enter_context(tc.tile_pool(name="expp", bufs=5))
        accp = ctx.enter_context(tc.tile_pool(name="accp", bufs=4))
        recp = ctx.enter_context(tc.tile_pool(name="recp", bufs=2))
        otp = ctx.enter_context(tc.tile_pool(name="otp", bufs=1))
        outp = ctx.enter_context(tc.tile_pool(name="outp", bufs=3))
        tmpp = ctx.enter_context(tc.tile_pool(name="tmpp", bufs=4))
        stp = ctx.enter_context(tc.tile_pool(name="stp", bufs=4))
        wop = ctx.enter_context(tc.tile_pool(name="wop", bufs=2))
        psA = ctx.enter_context(tc.tile_pool(name="psA", bufs=3, space="PSUM"))
        psOT = ctx.enter_context(tc.tile_pool(name="psOT", bufs=2,
                                              space="PSUM"))
        psSC = ctx.enter_context(tc.tile_pool(name="psSC", bufs=3,
                                              space="PSUM"))

        # resident weights
        wq_sb = wpool.tile([128, 32, FPC], BF, tag="wq")
        nc.sync.dma_start(wq_sb[:], wqT.rearrange("(o p) f -> p o f", p=128))
        wk_sb = wpool.tile([128, 32, FPC], BF, tag="wk")
        nc.sync.dma_start(wk_sb[:], wkT.rearrange("(o p) f -> p o f", p=128))
        wv_sb = wpool.tile([128, 32, FPC], BF, tag="wv")
        nc.sync.dma_start(wv_sb[:], wvT.rearrange("(o p) f -> p o f", p=128))
        if causal:
            mask_sb = wpool.tile([128, 4, 128], BF, tag="mask")
            nc.sync.dma_start(mask_sb[:], maskp.rearrange("j p n -> p j n"))
        else:
            mask_sb = wpool.tile([128, 8, 2, 512], F32, tag="mask")
            nc.sync.dma_start(mask_sb[:],
                              maskp.rearrange("j p q n -> p j q n"))
        ones_m = wpool.tile([128, 128], BF, tag="onesm")
        nc.gpsimd.memset(ones_m[:], 1.0)

        for b in range(B):
            Q_sb = qkvp.tile([128, 4, S], BF, tag="Qsb")
            K_sb = qkvp.tile([128, 4, S], BF, tag="Ksb")
            V_sb = qkvp.tile([128, 8, FPC], BF, tag="Vsb")
            OT_sb = otp.tile([128, 4, S], BF, tag="OT")

            def attn_half(qh):
                q0 = qh * 512
                kbs = list(range(0, qh * 4 + 4)) if causal else list(range(8))
                nkb = len(kbs)

                def width(kb):
                    # live query columns [qlo, 512) for this key block
                    if causal and kb - 4 * qh >= 0:
                        return (kb - 4 * qh) * 128
                    return 0

                for hp in range(2):
                    heads = (2 * hp, 2 * hp + 1)
                    ps_ot = {}
                    acc = {}
                    e_t = {}
                    for l in heads:
                        ps_ot[l] = psOT.tile([128, 512], F32, tag="psot", name="ps_ot")
                        acc[l] = accp.tile([128, 512], BF, tag="acc", name="acc")

                    def emit_sc(l, j):
                        kb = kbs[j]
                        qlo = width(kb)
                        ps_sc = psSC.tile([128, 512], F32, tag="pssc")
                        nc.tensor.matmul(
                            ps_sc[:, qlo:512], K_sb[:, l, kb * 128:kb * 128 + 128],
                            Q_sb[:, l, q0 + qlo:q0 + 512],
                            start=True, stop=True)
                        e_sb = expp.tile([128, 512], BF, tag="e")
                        if causal:
                            nc.scalar.activation(
                                e_sb[:, qlo:512], ps_sc[:, qlo:512],
                                mybir.ActivationFunctionType.Exp, scale=SCALE)
                            jj = kb - 4 * qh
                            if jj >= 0:
                                nc.vector.tensor_mul(
                                    e_sb[:, qlo:qlo + 128],
                                    e_sb[:, qlo:qlo + 128],
                                    mask_sb[:, jj, :])
                        else:
                            nc.vector.tensor_add(
                                ps_sc[:], ps_sc[:], mask_sb[:, kb, qh, :])
                            nc.scalar.activation(
                                e_sb[:], ps_sc[:],
                                mybir.ActivationFunctionType.Exp, scale=SCALE)
                        e_t[(l, j)] = e_sb

                    def emit_pv(l, j):
                        kb = kbs[j]
                        qlo = width(kb)
                        e_sb = e_t.pop((l, j))
                        nc.tensor.matmul(
                            ps_ot[l][:, qlo:512],
                            V_sb[:, kb, l * 128:(l + 1) * 128],
                            e_sb[:, qlo:512],
                            start=(j == 0), stop=(j == nkb - 1))
                        if j == 0:
                            nc.gpsimd.tensor_copy(acc[l][:], e_sb[:])
                        else:
                            nc.gpsimd.tensor_add(
                                acc[l][:, qlo:512], acc[l][:, qlo:512],
                                e_sb[:, qlo:512])

                    # software pipeline: PV lags one key block behind scores
                    for j in range(nkb):
                        for l in heads:
                            emit_sc(l, j)
                            if j > 0:
                                emit_pv(l, j - 1)
                    for l in heads:
                        emit_pv(l, nkb - 1)

                    # normalization tail for this head pair
                    rec = {}
                    for l in heads:
                        ps_den = psSC.tile([128, 512], F32, tag="pssc")
                        nc.tensor.matmul(ps_den[:], ones_m[:], acc[l][:],
                                         start=True, stop=True)
                        rec_bc = recp.tile([128, 512], F32, tag="rec")
                        nc.vector.reciprocal_approx_fast(rec_bc[:], ps_den[:])
                        rec[l] = rec_bc
                    for l in heads:
                        nc.vector.tensor_mul(OT_sb[:, l, q0:q0 + 512],
                                             ps_ot[l][:], rec[l][:])

            # ---- stage A: projections + RoPE ----
            for tt in range(NTT):
                t0 = b * S + tt * TT
                x_sb = xpool.tile([128, 32, TT], BF, tag="x")
                nc.sync.dma_start(
                    x_sb[:],
                    xT.rearrange("(o p) t -> p o t", p=128)[:, :, t0:t0 + TT])
                cc_sb = ccp.tile([128, TT], F32, tag="cc")
                nc.sync.dma_start(cc_sb[:], cc[:, t0:t0 + TT])
                ss_sb = ccp.tile([128, TT], F32, tag="ss")
                nc.sync.dma_start(ss_sb[:], ss[:, t0:t0 + TT])

                # Q and K, RoPE'd into Q_sb/K_sb
                for dst_sb, w_sb in ((Q_sb, wq_sb), (K_sb, wk_sb)):
                    for pair in range(2):
                        ps_pair = []
                        for ri in range(2):
                            f0 = pair * 256 + ri * 128
                            ps = psA.tile([128, 512], F32, tag="psa")
                            for d in range(32):
                                nc.tensor.matmul(
                                    ps[:, 0:TT], w_sb[:, d, f0:f0 + 128],
                                    x_sb[:, d, :], start=(d == 0),
                                    stop=(d == 31))
                            ps_pair.append(ps)
                        ps_r, ps_i = ps_pair
                        toff = tt * TT
                        t1 = tmpp.tile([128, TT], F32, tag="t")
                        nc.vector.tensor_mul(t1[:], ps_r[:, 0:TT], cc_sb[:])
                        t2 = tmpp.tile([128, TT], F32, tag="t")
                        nc.vector.tensor_mul(t2[:], ps_i[:, 0:TT], ss_sb[:])
                        st_r = stp.tile([128, TT], BF, tag="st")
                        nc.vector.tensor_tensor(
                            st_r[:], t1[:], t2[:], mybir.AluOpType.subtract)
                        t3 = tmpp.tile([128, TT], F32, tag="t")
                        nc.vector.tensor_mul(t3[:], ps_r[:, 0:TT], ss_sb[:])
                        t4 = tmpp.tile([128, TT], F32, tag="t")
                        nc.vector.tensor_mul(t4[:], ps_i[:, 0:TT], cc_sb[:])
                        st_i = stp.tile([128, TT], BF, tag="st")
                        nc.vector.tensor_tensor(
                            st_i[:], t3[:], t4[:], mybir.AluOpType.add)
                        # shuffle into head-contiguous blocks: head h of this
                        # pair = [r half; i half] on partitions [0:64|64:128]
                        for hh in range(2):
                            h_loc = 2 * pair + hh
                            nc.sync.dma_start(
                                dst_sb[0:64, h_loc, toff:toff + TT],
                                st_r[hh * 64:(hh + 1) * 64, :])
                            nc.sync.dma_start(
                                dst_sb[64:128, h_loc, toff:toff + TT],
                                st_i[hh * 64:(hh + 1) * 64, :])

                # V natural: per 128-token block
                for v in range(TT // 128):
                    tb = tt * (TT // 128) + v
                    ps_v = psA.tile([128, 512], F32, tag="psa")
                    for d in range(32):
                        nc.tensor.matmul(
                            ps_v[:], x_sb[:, d, v * 128:(v + 1) * 128],
                            wv_sb[:, d, :], start=(d == 0), stop=(d == 31))
                    nc.scalar.copy(V_sb[:, tb, :], ps_v[:])

                if tt == 1:
                    attn_half(0)
            attn_half(1)

            # ---- stage C: wo ----
            for nt in range(8):
                wo_sb = wop.tile([128, 4, 512], BF, tag="wo")
                nc.sync.dma_start(
                    wo_sb[:],
                    woT.rearrange("(o p) n -> p o n",
                                  p=128)[:, :, nt * 512:(nt + 1) * 512])
                for tb in range(8):
                    ps_o = psA.tile([128, 512], F32, tag="psa")
                    for k in range(4):
                        nc.tensor.matmul(
                            ps_o[:], OT_sb[:, k, tb * 128:(tb + 1) * 128],
                            wo_sb[:, k, :], start=(k == 0), stop=(k == 3))
                    o_sb = outp.tile([128, 512], F32, tag="o")
                    if tb % 2 == 0:
                        nc.scalar.copy(o_sb[:], ps_o[:])
                    else:
                        nc.vector.tensor_copy(o_sb[:], ps_o[:])
                    nc.sync.dma_start(
                        y[b * S + tb * 128:b * S + (tb + 1) * 128,
                          nt * 512:(nt + 1) * 512], o_sb[:])

    nc.compile()
    return nc


_CACHE = {}


def _get_program(causal):
    if causal not in _CACHE:
        _CACHE[causal] = _build_program(causal)
    return _CACHE[causal]


def kernel(x, wq_w, wq_a, wq_b, wk_w, wv_w, wv_a, wv_b, wo_w,
           freqs_cos, freqs_sin, mask, start_pos=0, _trace=False):
    assert int(np.asarray(start_pos)) == 0
    shared, cores, causal = _host_prep(
        x, wq_w, wq_a, wq_b, wk_w, wv_w, wv_a, wv_b, wo_w,
        freqs_cos, freqs_sin, mask)
    nc = _get_program(causal)
    in_maps = []
    for c in range(N_CORES):
        m = dict(xT=shared["xT"], cc=shared["cc"], ss=shared["ss"],
                 maskp=shared["maskp"])
        m.update(cores[c])
        in_maps.append(m)
    res = run_bass_kernel_spmd(nc, in_maps, list(range(N_CORES)),
                               trace=_trace)
    kernel._last_results = res
    acc = np.zeros((T, D), np.float32)
    for c in range(N_CORES):
        acc += np.asarray(res.results[c]["y"], np.float32)
    out = acc.reshape(B, S, D)
    return out.astype(np.asarray(x).dtype, copy=False)


# revision 6
# speedup vs baseline: 1.3885x; 1.1298x over previous
"""Trainium2 Bass kernel for LoRA attention prefill (B=4, S=1024, D=4096, H=32).

Sharding: tensor-parallel over heads. Each of the 8 cores computes 4 heads
(512 of the 4096 q/k/v features, column-shard of wq/wk/wv) and a row-shard
of wo, producing a full-shape [T, D] partial output; partials are summed on
the host.

v3 design notes:
  - LoRA folded into wq/wv on the host (exact algebra) - no device LoRA work.
  - Causal masking is multiplicative (0/1 bf16 after exp); diagonal score
    blocks only compute the live query range (partial-width matmuls).
  - Softmax denominators: exp tiles accumulated on DVE into a bf16 SBUF acc;
    one ones-matrix matmul per head broadcasts the denominator to all 128
    partitions; reciprocal_approx_fast (DVE) replaces the slow serial
    RECIPROCAL.
  - The PE instruction stream is software-pipelined end to end: attention
    rounds (which are exp/Scalar-latency bound) are interleaved with stage-A
    projection and stage-C wo matmul chunks via generators, so the in-order
    PE queue always has independent work. PV matmuls lag two rounds behind
    their score matmuls.
  - Startup weight DMAs are split into consumption-order chunks; the next
    batch's first x tile is prefetched before stage C.
"""
import sys
from contextlib import ExitStack

sys.path.insert(0, "/opt/trn_rl_repo")

import numpy as np
import ml_dtypes

import concourse.bass as bass
import concourse.mybir as mybir
import concourse.tile as tile
from concourse import bacc
from concourse.bass_utils import run_bass_kernel_spmd
from concourse.tile import TileContext

B, S, D = 4, 1024, 4096
H, HD = 32, 128
R = 16
LORA_SCALE = 2.0
N_CORES = 8
HPC = H // N_CORES            # heads per core
FPC = HPC * HD                # features per core = 512
T = B * S                     # 4096 tokens
TT = 256                      # stage-A T-tile (tokens)
NTT = S // TT                 # T-tiles per batch = 4
SCALE = float(1.0 / np.sqrt(HD))
BF = mybir.dt.bfloat16
F32 = mybir.dt.float32


def _bf(a):
    return np.ascontiguousarray(np.asarray(a, np.float32).astype(ml_dtypes.bfloat16))


def _core_perm(c):
    hs = [HPC * c + i for i in range(HPC)]
    ev = np.arange(0, HD, 2)
    od = np.arange(1, HD, 2)
    out = []
    for pair in (0, 1):
        h0, h1 = hs[2 * pair], hs[2 * pair + 1]
        out.append(h0 * HD + ev)
        out.append(h1 * HD + ev)
        out.append(h0 * HD + od)
        out.append(h1 * HD + od)
    return np.concatenate(out)


def _check_causal(mask):
    iu = np.triu_indices(S, k=1)
    il = np.tril_indices(S, k=0)
    return bool((mask[iu] <= -1e8).all() and (mask[il] == 0).all())


def _host_prep(x, wq_w, wq_a, wq_b, wk_w, wv_w, wv_a, wv_b, wo_w,
               freqs_cos, freqs_sin, mask):
    x2 = np.asarray(x, np.float32).reshape(T, D)
    xT = _bf(x2.T)

    # fold LoRA into the dense weights: y = x(W + s*B@A)^T exactly
    wq_eff = np.asarray(wq_w, np.float32) + LORA_SCALE * (
        np.asarray(wq_b, np.float32) @ np.asarray(wq_a, np.float32))
    wv_eff = np.asarray(wv_w, np.float32) + LORA_SCALE * (
        np.asarray(wv_b, np.float32) @ np.asarray(wv_a, np.float32))
    wk = np.asarray(wk_w, np.float32)

    cosT = np.asarray(freqs_cos, np.float32).T
    sinT = np.asarray(freqs_sin, np.float32).T
    cc = np.ascontiguousarray(np.tile(cosT, (2, B)).astype(np.float32))
    ss = np.ascontiguousarray(np.tile(sinT, (2, B)).astype(np.float32))

    mask = np.asarray(mask, np.float32)
    causal = _check_causal(mask)
    if causal:
        # 0/1 multiplicative triangle for the 128x128 diagonal strips
        tri = np.tril(np.ones((128, 128), np.float32)).T  # [k,q]: 1 if k<=q
        maskp = _bf(np.broadcast_to(tri, (4, 128, 128)))
    else:
        mT = mask.T * np.float32(np.sqrt(HD))
        maskp = np.zeros((8, 128, 2, 512), np.float32)
        for qh in range(2):
            for j in range(8):
                maskp[j, :, qh, :] = mT[j * 128:(j + 1) * 128,
                                        qh * 512:(qh + 1) * 512]

    shared = dict(xT=xT, cc=cc, ss=ss, maskp=maskp)
    cores = []
    for c in range(N_CORES):
        perm = _core_perm(c)
        sl = slice(c * FPC, (c + 1) * FPC)
        cores.append(dict(
            wqT=_bf(wq_eff[perm, :].T),
            wkT=_bf(wk[perm, :].T),
            wvT=_bf(wv_eff[sl, :].T),
            woT=_bf(np.asarray(wo_w, np.float32)[:, sl].T),
        ))
    return shared, cores, causal


def _zip_chunks(*gens):
    """Round-robin drive generators to completion."""
    gens = list(gens)
    while gens:
        for g in list(gens):
            try:
                next(g)
            except StopIteration:
                gens.remove(g)


def _build_program(causal):
    nc = bacc.Bacc("TRN2", num_devices=N_CORES)

    xT = nc.dram_tensor("xT", [D, T], BF, kind="ExternalInput").ap()
    wqT = nc.dram_tensor("wqT", [D, FPC], BF, kind="ExternalInput").ap()
    wkT = nc.dram_tensor("wkT", [D, FPC], BF, kind="ExternalInput").ap()
    wvT = nc.dram_tensor("wvT", [D, FPC], BF, kind="ExternalInput").ap()
    woT = nc.dram_tensor("woT", [FPC, D], BF, kind="ExternalInput").ap()
    cc = nc.dram_tensor("cc", [128, T], F32, kind="ExternalInput").ap()
    ss = nc.dram_tensor("ss", [128, T], F32, kind="ExternalInput").ap()
    if causal:
        maskp = nc.dram_tensor("maskp", [4, 128, 128], BF,
                               kind="ExternalInput").ap()
    else:
        maskp = nc.dram_tensor("maskp", [8, 128, 2, 512], F32,
                               kind="ExternalInput").ap()
    y = nc.dram_tensor("y", [T, D], F32, kind="ExternalOutput").ap()

    with TileContext(nc) as tc, ExitStack() as ctx:
        wpool = ctx.enter_context(tc.tile_pool(name="wpool", bufs=1))
        xpool = ctx.enter_context(tc.tile_pool(name="xpool", bufs=2))
        ccp = ctx.enter_context(tc.tile_pool(name="ccp", bufs=4))
        qkvp = ctx.enter_context(tc.tile_pool(name="qkvp", bufs=1))
        expp = ctx.enter_context(tc.tile_pool(name="expp", bufs=7))
        accp = ctx.enter_context(tc.tile_pool(name="accp", bufs=4))
        recp = ctx.enter_context(tc.tile_pool(name="recp", bufs=2))
        otp = ctx.enter_context(tc.tile_pool(name="otp", bufs=1))
        outp = ctx.enter_context(tc.tile_pool(name="outp", bufs=3))
        tmpp = ctx.enter_context(tc.tile_pool(name="tmpp", bufs=4))
        stp = ctx.enter_context(tc.tile_pool(name="stp", bufs=4))
        wop = ctx.enter_context(tc.tile_pool(name="wop", bufs=2))
        psA = ctx.enter_context(tc.tile_pool(name="psA", bufs=3, space="PSUM"))
        psOT = ctx.enter_context(tc.tile_pool(name="psOT", bufs=2,
                                              space="PSUM"))
        psSC = ctx.enter_context(tc.tile_pool(name="psSC", bufs=3,
                                              space="PSUM"))

        # resident weights, split into consumption-order chunks so the first
        # projection matmuls can start before the full weight set has landed
        wq_sb = wpool.tile([128, 32, FPC], BF, tag="wq")
        wk_sb = wpool.tile([128, 32, FPC], BF, tag="wk")
        wv_sb = wpool.tile([128, 32, FPC], BF, tag="wv")
        wqr = wqT.rearrange("(o p) f -> p o f", p=128)
        wkr = wkT.rearrange("(o p) f -> p o f", p=128)
        wvr = wvT.rearrange("(o p) f -> p o f", p=128)
        for f0 in range(0, FPC, 128):
            nc.sync.dma_start(wq_sb[:, :, f0:f0 + 128], wqr[:, :, f0:f0 + 128])
        for f0 in range(0, FPC, 128):
            nc.sync.dma_start(wk_sb[:, :, f0:f0 + 128], wkr[:, :, f0:f0 + 128])
        for f0 in range(0, FPC, 256):
            nc.sync.dma_start(wv_sb[:, :, f0:f0 + 256], wvr[:, :, f0:f0 + 256])
        if causal:
            mask_sb = wpool.tile([128, 4, 128], BF, tag="mask")
            nc.sync.dma_start(mask_sb[:], maskp.rearrange("j p n -> p j n"))
        else:
            mask_sb = wpool.tile([128, 8, 2, 512], F32, tag="mask")
            nc.sync.dma_start(mask_sb[:],
                              maskp.rearrange("j p q n -> p j q n"))
        ones_m = wpool.tile([128, 128], BF, tag="onesm")
        nc.gpsimd.memset(ones_m[:], 1.0)

        xre = xT.rearrange("(o p) t -> p o t", p=128)
        # prefetched stage-A input tiles, keyed by (b, tt)
        fetched = {}

        def fetch_x(b, tt):
            t0 = b * S + tt * TT
            x_sb = xpool.tile([128, 32, TT], BF, tag="x", name="x_sb")
            nc.sync.dma_start(x_sb[:], xre[:, :, t0:t0 + TT])
            cc_sb = ccp.tile([128, TT], F32, tag="cc", name="cc_sb")
            nc.sync.dma_start(cc_sb[:], cc[:, t0:t0 + TT])
            ss_sb = ccp.tile([128, TT], F32, tag="ss", name="ss_sb")
            nc.sync.dma_start(ss_sb[:], ss[:, t0:t0 + TT])
            fetched[(b, tt)] = (x_sb, cc_sb, ss_sb)

        for b in range(B):
            Q_sb = qkvp.tile([128, 4, S], BF, tag="Qsb")
            K_sb = qkvp.tile([128, 4, S], BF, tag="Ksb")
            V_sb = qkvp.tile([128, 8, FPC], BF, tag="Vsb")
            OT_sb = otp.tile([128, 4, S], BF, tag="OT")

            def stage_a(tt, prefetch_next):
                """Generator: QK pair groups + V blocks for one t-tile."""
                x_sb, cc_sb, ss_sb = fetched.pop((b, tt))
                if prefetch_next is not None:
                    fetch_x(*prefetch_next)
                toff = tt * TT
                for dst_sb, w_sb in ((Q_sb, wq_sb), (K_sb, wk_sb)):
                    for pair in range(2):
                        ps_pair = []
                        for ri in range(2):
                            f0 = pair * 256 + ri * 128
                            ps = psA.tile([128, 512], F32, tag="psa",
                                          name="ps_qk")
                            for d in range(32):
                                nc.tensor.matmul(
                                    ps[:, 0:TT], w_sb[:, d, f0:f0 + 128],
                                    x_sb[:, d, :], start=(d == 0),
                                    stop=(d == 31))
                            ps_pair.append(ps)
                        ps_r, ps_i = ps_pair
                        t1 = tmpp.tile([128, TT], F32, tag="t", name="t1")
                        nc.vector.tensor_mul(t1[:], ps_r[:, 0:TT], cc_sb[:])
                        t2 = tmpp.tile([128, TT], F32, tag="t", name="t2")
                        nc.vector.tensor_mul(t2[:], ps_i[:, 0:TT], ss_sb[:])
                        st_r = stp.tile([128, TT], BF, tag="st", name="st_r")
                        nc.vector.tensor_tensor(
                            st_r[:], t1[:], t2[:], mybir.AluOpType.subtract)
                        t3 = tmpp.tile([128, TT], F32, tag="t", name="t3")
                        nc.vector.tensor_mul(t3[:], ps_r[:, 0:TT], ss_sb[:])
                        t4 = tmpp.tile([128, TT], F32, tag="t", name="t4")
                        nc.vector.tensor_mul(t4[:], ps_i[:, 0:TT], cc_sb[:])
                        st_i = stp.tile([128, TT], BF, tag="st", name="st_i")
                        nc.vector.tensor_tensor(
                            st_i[:], t3[:], t4[:], mybir.AluOpType.add)
                        # shuffle into head-contiguous blocks: head h of this
                        # pair = [r half; i half] on partitions [0:64|64:128]
                        for hh in range(2):
                            h_loc = 2 * pair + hh
                            nc.sync.dma_start(
                                dst_sb[0:64, h_loc, toff:toff + TT],
                                st_r[hh * 64:(hh + 1) * 64, :])
                            nc.sync.dma_start(
                                dst_sb[64:128, h_loc, toff:toff + TT],
                                st_i[hh * 64:(hh + 1) * 64, :])
                        yield
                for v in range(TT // 128):
                    tb = tt * (TT // 128) + v
                    ps_v = psA.tile([128, 512], F32, tag="psa", name="ps_v")
                    for d in range(32):
                        nc.tensor.matmul(
                            ps_v[:], x_sb[:, d, v * 128:(v + 1) * 128],
                            wv_sb[:, d, :], start=(d == 0), stop=(d == 31))
                    nc.scalar.copy(V_sb[:, tb, :], ps_v[:])
                    yield

            def attn_half(qh):
                """Generator: attention rounds for one query half."""
                q0 = qh * 512
                kbs = list(range(0, qh * 4 + 4)) if causal else list(range(8))
                nkb = len(kbs)
                LAG = 2

                def width(kb):
                    if causal and kb - 4 * qh >= 0:
                        return (kb - 4 * qh) * 128
                    return 0

                for hp in range(2):
                    heads = (2 * hp, 2 * hp + 1)
                    ps_ot = {}
                    acc = {}
                    e_t = {}
                    for l in heads:
                        ps_ot[l] = psOT.tile([128, 512], F32, tag="psot",
                                             name="ps_ot")
                        acc[l] = accp.tile([128, 512], BF, tag="acc",
                                           name="acc")

                    def emit_sc(l, j):
                        kb = kbs[j]
                        qlo = width(kb)
                        ps_sc = psSC.tile([128, 512], F32, tag="pssc",
                                          name="ps_sc")
                        nc.tensor.matmul(
                            ps_sc[:, qlo:512],
                            K_sb[:, l, kb * 128:kb * 128 + 128],
                            Q_sb[:, l, q0 + qlo:q0 + 512],
                            start=True, stop=True)
                        e_sb = expp.tile([128, 512], BF, tag="e", name="e_sb")
                        if causal:
                            nc.scalar.activation(
                                e_sb[:, qlo:512], ps_sc[:, qlo:512],
                                mybir.ActivationFunctionType.Exp, scale=SCALE)
                            jj = kb - 4 * qh
                            if jj >= 0:
                                nc.vector.tensor_mul(
                                    e_sb[:, qlo:qlo + 128],
                                    e_sb[:, qlo:qlo + 128],
                                    mask_sb[:, jj, :])
                        else:
                            nc.vector.tensor_add(
                                ps_sc[:], ps_sc[:], mask_sb[:, kb, qh, :])
                            nc.scalar.activation(
                                e_sb[:], ps_sc[:],
                                mybir.ActivationFunctionType.Exp, scale=SCALE)
                        e_t[(l, j)] = e_sb

                    def emit_pv(l, j):
                        kb = kbs[j]
                        qlo = width(kb)
                        e_sb = e_t.pop((l, j))
                        nc.tensor.matmul(
                            ps_ot[l][:, qlo:512],
                            V_sb[:, kb, l * 128:(l + 1) * 128],
                            e_sb[:, qlo:512],
                            start=(j == 0), stop=(j == nkb - 1))
                        if j == 0:
                            nc.vector.tensor_copy(acc[l][:], e_sb[:])
                        else:
                            nc.vector.tensor_add(
                                acc[l][:, qlo:512], acc[l][:, qlo:512],
                                e_sb[:, qlo:512])

                    # software pipeline: PV lags LAG key blocks behind scores
                    for j in range(nkb):
                        for l in heads:
                            emit_sc(l, j)
                            if j >= LAG:
                                emit_pv(l, j - LAG)
                        yield
                    for j in range(max(nkb - LAG, 0), nkb):
                        for l in heads:
                            emit_pv(l, j)
                    # normalization tail for this head pair
                    rec = {}
                    for l in heads:
                        ps_den = psSC.tile([128, 512], F32, tag="pssc",
                                           name="ps_den")
                        nc.tensor.matmul(ps_den[:], ones_m[:], acc[l][:],
                                         start=True, stop=True)
                        rec_bc = recp.tile([128, 512], F32, tag="rec",
                                           name="rec_bc")
                        nc.vector.reciprocal_approx_fast(rec_bc[:], ps_den[:])
                        rec[l] = rec_bc
                    for l in heads:
                        nc.vector.tensor_mul(OT_sb[:, l, q0:q0 + 512],
                                             ps_ot[l][:], rec[l][:])
                    yield

            def stage_c(tbs):
                """Generator: wo matmuls for the given token blocks."""
                for nt in range(8):
                    wo_sb = wop.tile([128, 4, 512], BF, tag="wo",
                                     name="wo_sb")
                    nc.sync.dma_start(
                        wo_sb[:],
                        woT.rearrange("(o p) n -> p o n",
                                      p=128)[:, :, nt * 512:(nt + 1) * 512])
                    for i, tb in enumerate(tbs):
                        ps_o = psA.tile([128, 512], F32, tag="psa",
                                        name="ps_o")
                        for k in range(4):
                            nc.tensor.matmul(
                                ps_o[:], OT_sb[:, k, tb * 128:(tb + 1) * 128],
                                wo_sb[:, k, :], start=(k == 0), stop=(k == 3))
                        o_sb = outp.tile([128, 512], F32, tag="o",
                                         name="o_sb")
                        if tb % 2 == 0:
                            nc.scalar.copy(o_sb[:], ps_o[:])
                        else:
                            nc.vector.tensor_copy(o_sb[:], ps_o[:])
                        nc.sync.dma_start(
                            y[b * S + tb * 128:b * S + (tb + 1) * 128,
                              nt * 512:(nt + 1) * 512], o_sb[:])
                        if i % 2 == 1:
                            yield
                    yield

            # ---- batch schedule ----
            if b == 0:
                fetch_x(0, 0)
            # tt=0 prefetches tt=1, etc.
            for _ in stage_a(0, (b, 1)):
                pass
            for _ in stage_a(1, (b, 2)):
                pass
            # attention qh=0 zipped with stage A tt=2,3
            def a_tail():
                for u in stage_a(2, (b, 3)):
                    yield u
                nxt = (b + 1, 0) if b + 1 < B else None
                for u in stage_a(3, nxt):
                    yield u
            _zip_chunks(a_tail(), attn_half(0))
            # attention qh=1 zipped with stage C for its ready token blocks
            _zip_chunks(stage_c([0, 1, 2, 3]), attn_half(1))
            for _ in stage_c([4, 5, 6, 7]):
                pass

    nc.compile()
    return nc


_CACHE = {}


def _get_program(causal):
    if causal not in _CACHE:
        _CACHE[causal] = _build_program(causal)
    return _CACHE[causal]


def kernel(x, wq_w, wq_a, wq_b, wk_w, wv_w, wv_a, wv_b, wo_w,
           freqs_cos, freqs_sin, mask, start_pos=0, _trace=False):
    assert int(np.asarray(start_pos)) == 0
    shared, cores, causal = _host_prep(
        x, wq_w, wq_a, wq_b, wk_w, wv_w, wv_a, wv_b, wo_w,
        freqs_cos, freqs_sin, mask)
    nc = _get_program(causal)
    in_maps = []
    for c in range(N_CORES):
        m = dict(xT=shared["xT"], cc=shared["cc"], ss=shared["ss"],
                 maskp=shared["maskp"])
        m.update(cores[c])
        in_maps.append(m)
    res = run_bass_kernel_spmd(nc, in_maps, list(range(N_CORES)),
                               trace=_trace)
    kernel._last_results = res
    acc = np.zeros((T, D), np.float32)
    for c in range(N_CORES):
        acc += np.asarray(res.results[c]["y"], np.float32)
    out = acc.reshape(B, S, D)
    return out.astype(np.asarray(x).dtype, copy=False)


# revision 10
# speedup vs baseline: 1.4777x; 1.0642x over previous
"""Trainium2 Bass kernel for LoRA attention prefill (B=4, S=1024, D=4096, H=32).

Sharding: tensor-parallel over heads. Each of the 8 cores computes 4 heads
(512 of the 4096 q/k/v features, column-shard of wq/wk/wv) and a row-shard
of wo, producing a full-shape [T, D] partial output; partials are summed on
the host.

v3 design notes:
  - LoRA folded into wq/wv on the host (exact algebra) - no device LoRA work.
  - Causal masking is multiplicative (0/1 bf16 after exp); diagonal score
    blocks only compute the live query range (partial-width matmuls).
  - Softmax denominators: exp tiles accumulated on DVE into a bf16 SBUF acc;
    one ones-matrix matmul per head broadcasts the denominator to all 128
    partitions; reciprocal_approx_fast (DVE) replaces the slow serial
    RECIPROCAL.
  - The PE instruction stream is software-pipelined end to end: attention
    rounds (which are exp/Scalar-latency bound) are interleaved with stage-A
    projection and stage-C wo matmul chunks via generators, so the in-order
    PE queue always has independent work. PV matmuls lag two rounds behind
    their score matmuls.
  - Startup weight DMAs are split into consumption-order chunks; the next
    batch's first x tile is prefetched before stage C.
"""
import sys
from contextlib import ExitStack

sys.path.insert(0, "/opt/trn_rl_repo")

import numpy as np
import ml_dtypes

import concourse.bass as bass
import concourse.mybir as mybir
import concourse.tile as tile
from concourse import bacc
from concourse.bass_utils import run_bass_kernel_spmd
from concourse.tile import TileContext

B, S, D = 4, 1024, 4096
H, HD = 32, 128
R = 16
LORA_SCALE = 2.0
N_CORES = 8
HPC = H // N_CORES            # heads per core
FPC = HPC * HD                # features per core = 512
T = B * S                     # 4096 tokens
TT = 256                      # stage-A T-tile (tokens)
NTT = S // TT                 # T-tiles per batch = 4
SCALE = float(1.0 / np.sqrt(HD))
BF = mybir.dt.bfloat16
F32 = mybir.dt.float32


def _bf(a):
    return np.ascontiguousarray(np.asarray(a, np.float32).astype(ml_dtypes.bfloat16))


def _core_perm(c):
    hs = [HPC * c + i for i in range(HPC)]
    ev = np.arange(0, HD, 2)
    od = np.arange(1, HD, 2)
    out = []
    for pair in (0, 1):
        h0, h1 = hs[2 * pair], hs[2 * pair + 1]
        out.append(h0 * HD + ev)
        out.append(h1 * HD + ev)
        out.append(h0 * HD + od)
        out.append(h1 * HD + od)
    return np.concatenate(out)


def _check_causal(mask):
    iu = np.triu_indices(S, k=1)
    il = np.tril_indices(S, k=0)
    return bool((mask[iu] <= -1e8).all() and (mask[il] == 0).all())


def _host_prep(x, wq_w, wq_a, wq_b, wk_w, wv_w, wv_a, wv_b, wo_w,
               freqs_cos, freqs_sin, mask):
    x2 = np.asarray(x, np.float32).reshape(T, D)
    xT = _bf(x2.T)

    # fold LoRA into the dense weights: y = x(W + s*B@A)^T exactly
    wq_eff = np.asarray(wq_w, np.float32) + LORA_SCALE * (
        np.asarray(wq_b, np.float32) @ np.asarray(wq_a, np.float32))
    wv_eff = np.asarray(wv_w, np.float32) + LORA_SCALE * (
        np.asarray(wv_b, np.float32) @ np.asarray(wv_a, np.float32))
    wk = np.asarray(wk_w, np.float32)

    cosT = np.asarray(freqs_cos, np.float32).T
    sinT = np.asarray(freqs_sin, np.float32).T
    cc = np.ascontiguousarray(np.tile(cosT, (2, B)).astype(np.float32))
    ss = np.ascontiguousarray(np.tile(sinT, (2, B)).astype(np.float32))

    mask = np.asarray(mask, np.float32)
    causal = _check_causal(mask)
    if causal:
        # 0/1 multiplicative triangle for the 128x128 diagonal strips
        tri = np.tril(np.ones((128, 128), np.float32)).T  # [k,q]: 1 if k<=q
        maskp = _bf(np.broadcast_to(tri, (4, 128, 128)))
    else:
        mT = mask.T * np.float32(np.sqrt(HD))
        maskp = np.zeros((8, 128, 2, 512), np.float32)
        for qh in range(2):
            for j in range(8):
                maskp[j, :, qh, :] = mT[j * 128:(j + 1) * 128,
                                        qh * 512:(qh + 1) * 512]

    shared = dict(xT=xT, cc=cc, ss=ss, maskp=maskp)
    cores = []
    for c in range(N_CORES):
        perm = _core_perm(c)
        sl = slice(c * FPC, (c + 1) * FPC)
        cores.append(dict(
            wqT=_bf(wq_eff[perm, :].T),
            wkT=_bf(wk[perm, :].T),
            wvT=_bf(wv_eff[sl, :].T),
            woT=_bf(np.asarray(wo_w, np.float32)[:, sl].T),
        ))
    return shared, cores, causal


def _zip_chunks(*gens):
    """Round-robin drive generators to completion."""
    gens = list(gens)
    while gens:
        for g in list(gens):
            try:
                next(g)
            except StopIteration:
                gens.remove(g)


def _build_program(causal):
    nc = bacc.Bacc("TRN2", num_devices=N_CORES)

    xT = nc.dram_tensor("xT", [D, T], BF, kind="ExternalInput").ap()
    wqT = nc.dram_tensor("wqT", [D, FPC], BF, kind="ExternalInput").ap()
    wkT = nc.dram_tensor("wkT", [D, FPC], BF, kind="ExternalInput").ap()
    wvT = nc.dram_tensor("wvT", [D, FPC], BF, kind="ExternalInput").ap()
    woT = nc.dram_tensor("woT", [FPC, D], BF, kind="ExternalInput").ap()
    cc = nc.dram_tensor("cc", [128, T], F32, kind="ExternalInput").ap()
    ss = nc.dram_tensor("ss", [128, T], F32, kind="ExternalInput").ap()
    if causal:
        maskp = nc.dram_tensor("maskp", [4, 128, 128], BF,
                               kind="ExternalInput").ap()
    else:
        maskp = nc.dram_tensor("maskp", [8, 128, 2, 512], F32,
                               kind="ExternalInput").ap()
    y = nc.dram_tensor("y", [T, D], F32, kind="ExternalOutput").ap()

    with TileContext(nc) as tc, ExitStack() as ctx:
        wpool = ctx.enter_context(tc.tile_pool(name="wpool", bufs=1))
        xpool = ctx.enter_context(tc.tile_pool(name="xpool", bufs=2))
        ccp = ctx.enter_context(tc.tile_pool(name="ccp", bufs=4))
        qkvp = ctx.enter_context(tc.tile_pool(name="qkvp", bufs=1))
        expp = ctx.enter_context(tc.tile_pool(name="expp", bufs=7))
        accp = ctx.enter_context(tc.tile_pool(name="accp", bufs=4))
        recp = ctx.enter_context(tc.tile_pool(name="recp", bufs=2))
        otp = ctx.enter_context(tc.tile_pool(name="otp", bufs=1))
        outp = ctx.enter_context(tc.tile_pool(name="outp", bufs=3))
        tmpp = ctx.enter_context(tc.tile_pool(name="tmpp", bufs=4))
        stp = ctx.enter_context(tc.tile_pool(name="stp", bufs=4))
        wop = ctx.enter_context(tc.tile_pool(name="wop", bufs=2))
        psA = ctx.enter_context(tc.tile_pool(name="psA", bufs=3, space="PSUM"))
        psOT = ctx.enter_context(tc.tile_pool(name="psOT", bufs=2,
                                              space="PSUM"))
        psSC = ctx.enter_context(tc.tile_pool(name="psSC", bufs=3,
                                              space="PSUM"))

        xre = xT.rearrange("(o p) t -> p o t", p=128)
        # prefetched stage-A input tiles, keyed by (b, tt)
        fetched = {}

        def fetch_x(b, tt):
            t0 = b * S + tt * TT
            x_sb = xpool.tile([128, 32, TT], BF, tag="x", name="x_sb")
            nc.sync.dma_start(x_sb[:], xre[:, :, t0:t0 + TT])
            cc_sb = ccp.tile([128, TT], F32, tag="cc", name="cc_sb")
            nc.sync.dma_start(cc_sb[:], cc[:, t0:t0 + TT])
            ss_sb = ccp.tile([128, TT], F32, tag="ss", name="ss_sb")
            nc.sync.dma_start(ss_sb[:], ss[:, t0:t0 + TT])
            fetched[(b, tt)] = (x_sb, cc_sb, ss_sb)

        # first input tile before the weights so compute starts early
        fetch_x(0, 0)

        # resident weights, split into consumption-order chunks so the first
        # projection matmuls can start before the full weight set has landed
        wq_sb = wpool.tile([128, 32, FPC], BF, tag="wq")
        wk_sb = wpool.tile([128, 32, FPC], BF, tag="wk")
        wv_sb = wpool.tile([128, 32, FPC], BF, tag="wv")
        wqr = wqT.rearrange("(o p) f -> p o f", p=128)
        wkr = wkT.rearrange("(o p) f -> p o f", p=128)
        wvr = wvT.rearrange("(o p) f -> p o f", p=128)
        for f0 in range(0, FPC, 128):
            nc.sync.dma_start(wq_sb[:, :, f0:f0 + 128], wqr[:, :, f0:f0 + 128])
        for f0 in range(0, FPC, 128):
            nc.sync.dma_start(wk_sb[:, :, f0:f0 + 128], wkr[:, :, f0:f0 + 128])
        for f0 in range(0, FPC, 256):
            nc.sync.dma_start(wv_sb[:, :, f0:f0 + 256], wvr[:, :, f0:f0 + 256])
        if causal:
            mask_sb = wpool.tile([128, 4, 128], BF, tag="mask")
            nc.sync.dma_start(mask_sb[:], maskp.rearrange("j p n -> p j n"))
        else:
            mask_sb = wpool.tile([128, 8, 2, 512], F32, tag="mask")
            nc.sync.dma_start(mask_sb[:],
                              maskp.rearrange("j p q n -> p j q n"))
        ones_m = wpool.tile([128, 128], BF, tag="onesm")
        nc.gpsimd.memset(ones_m[:], 1.0)

        pending_c_tail = None
        for b in range(B):
            Q_sb = qkvp.tile([128, 4, S], BF, tag="Qsb")
            K_sb = qkvp.tile([128, 4, S], BF, tag="Ksb")
            V_sb = qkvp.tile([128, 8, FPC], BF, tag="Vsb")
            OT_sb = otp.tile([128, 4, S], BF, tag="OT")

            def stage_a(tt, prefetch_next):
                """Generator: QK pair groups + V blocks for one t-tile."""
                x_sb, cc_sb, ss_sb = fetched.pop((b, tt))
                if prefetch_next is not None:
                    fetch_x(*prefetch_next)
                toff = tt * TT
                for dst_sb, w_sb in ((Q_sb, wq_sb), (K_sb, wk_sb)):
                    for pair in range(2):
                        ps_pair = []
                        for ri in range(2):
                            f0 = pair * 256 + ri * 128
                            ps = psA.tile([128, 512], F32, tag="psa",
                                          name="ps_qk")
                            for d in range(32):
                                nc.tensor.matmul(
                                    ps[:, 0:TT], w_sb[:, d, f0:f0 + 128],
                                    x_sb[:, d, :], start=(d == 0),
                                    stop=(d == 31))
                            ps_pair.append(ps)
                        ps_r, ps_i = ps_pair
                        t1 = tmpp.tile([128, TT], F32, tag="t", name="t1")
                        nc.vector.tensor_mul(t1[:], ps_r[:, 0:TT], cc_sb[:])
                        t2 = tmpp.tile([128, TT], F32, tag="t", name="t2")
                        nc.vector.tensor_mul(t2[:], ps_i[:, 0:TT], ss_sb[:])
                        st_r = stp.tile([128, TT], BF, tag="st", name="st_r")
                        nc.vector.tensor_tensor(
                            st_r[:], t1[:], t2[:], mybir.AluOpType.subtract)
                        t3 = tmpp.tile([128, TT], F32, tag="t", name="t3")
                        nc.vector.tensor_mul(t3[:], ps_r[:, 0:TT], ss_sb[:])
                        t4 = tmpp.tile([128, TT], F32, tag="t", name="t4")
                        nc.vector.tensor_mul(t4[:], ps_i[:, 0:TT], cc_sb[:])
                        st_i = stp.tile([128, TT], BF, tag="st", name="st_i")
                        nc.vector.tensor_tensor(
                            st_i[:], t3[:], t4[:], mybir.AluOpType.add)
                        # shuffle into head-contiguous blocks: head h of this
                        # pair = [r half; i half] on partitions [0:64|64:128]
                        for hh in range(2):
                            h_loc = 2 * pair + hh
                            nc.sync.dma_start(
                                dst_sb[0:64, h_loc, toff:toff + TT],
                                st_r[hh * 64:(hh + 1) * 64, :])
                            nc.sync.dma_start(
                                dst_sb[64:128, h_loc, toff:toff + TT],
                                st_i[hh * 64:(hh + 1) * 64, :])
                        yield
                for v in range(TT // 128):
                    tb = tt * (TT // 128) + v
                    ps_v = psA.tile([128, 512], F32, tag="psa", name="ps_v")
                    for d in range(32):
                        nc.tensor.matmul(
                            ps_v[:], x_sb[:, d, v * 128:(v + 1) * 128],
                            wv_sb[:, d, :], start=(d == 0), stop=(d == 31))
                    nc.scalar.copy(V_sb[:, tb, :], ps_v[:])
                    yield

            def attn_half(qh):
                """Generator: attention rounds for one query half."""
                q0 = qh * 512
                kbs = list(range(0, qh * 4 + 4)) if causal else list(range(8))
                nkb = len(kbs)
                LAG = 2

                def width(kb):
                    if causal and kb - 4 * qh >= 0:
                        return (kb - 4 * qh) * 128
                    return 0

                for hp in range(2):
                    heads = (2 * hp, 2 * hp + 1)
                    ps_ot = {}
                    acc = {}
                    e_t = {}
                    for l in heads:
                        ps_ot[l] = psOT.tile([128, 512], F32, tag="psot",
                                             name="ps_ot")
                        acc[l] = accp.tile([128, 512], BF, tag="acc",
                                           name="acc")

                    def emit_sc(l, j):
                        kb = kbs[j]
                        qlo = width(kb)
                        ps_sc = psSC.tile([128, 512], F32, tag="pssc",
                                          name="ps_sc")
                        nc.tensor.matmul(
                            ps_sc[:, qlo:512],
                            K_sb[:, l, kb * 128:kb * 128 + 128],
                            Q_sb[:, l, q0 + qlo:q0 + 512],
                            start=True, stop=True)
                        e_sb = expp.tile([128, 512], BF, tag="e", name="e_sb")
                        if causal:
                            nc.scalar.activation(
                                e_sb[:, qlo:512], ps_sc[:, qlo:512],
                                mybir.ActivationFunctionType.Exp, scale=SCALE)
                            jj = kb - 4 * qh
                            if jj >= 0:
                                nc.vector.tensor_mul(
                                    e_sb[:, qlo:qlo + 128],
                                    e_sb[:, qlo:qlo + 128],
                                    mask_sb[:, jj, :])
                        else:
                            nc.vector.tensor_add(
                                ps_sc[:], ps_sc[:], mask_sb[:, kb, qh, :])
                            nc.scalar.activation(
                                e_sb[:], ps_sc[:],
                                mybir.ActivationFunctionType.Exp, scale=SCALE)
                        e_t[(l, j)] = e_sb

                    def emit_pv(l, j):
                        kb = kbs[j]
                        qlo = width(kb)
                        e_sb = e_t.pop((l, j))
                        nc.tensor.matmul(
                            ps_ot[l][:, qlo:512],
                            V_sb[:, kb, l * 128:(l + 1) * 128],
                            e_sb[:, qlo:512],
                            start=(j == 0), stop=(j == nkb - 1))
                        if j == 0:
                            nc.vector.tensor_copy(acc[l][:], e_sb[:])
                        else:
                            nc.vector.tensor_add(
                                acc[l][:, qlo:512], acc[l][:, qlo:512],
                                e_sb[:, qlo:512])

                    # software pipeline: PV lags LAG key blocks behind scores
                    for j in range(nkb):
                        for l in heads:
                            emit_sc(l, j)
                            if j >= LAG:
                                emit_pv(l, j - LAG)
                        yield
                    for j in range(max(nkb - LAG, 0), nkb):
                        for l in heads:
                            emit_pv(l, j)
                    # normalization tail for this head pair
                    rec = {}
                    for l in heads:
                        ps_den = psSC.tile([128, 512], F32, tag="pssc",
                                           name="ps_den")
                        nc.tensor.matmul(ps_den[:], ones_m[:], acc[l][:],
                                         start=True, stop=True)
                        rec_bc = recp.tile([128, 512], F32, tag="rec",
                                           name="rec_bc")
                        nc.vector.reciprocal_approx_fast(rec_bc[:], ps_den[:])
                        rec[l] = rec_bc
                    for l in heads:
                        nc.vector.tensor_mul(OT_sb[:, l, q0:q0 + 512],
                                             ps_ot[l][:], rec[l][:])
                    yield

            def stage_c(tbs, pools, b=b, OT_sb=OT_sb):
                """Generator: wo matmuls for the given token blocks.

                pools: PSUM pools to rotate ps_o through (pass the idle
                attention pools for the cross-batch tail so drain latency is
                fully hidden)."""
                npool = 0
                for nt in range(8):
                    wo_sb = wop.tile([128, 4, 512], BF, tag="wo",
                                     name="wo_sb")
                    nc.sync.dma_start(
                        wo_sb[:],
                        woT.rearrange("(o p) n -> p o n",
                                      p=128)[:, :, nt * 512:(nt + 1) * 512])
                    for i, tb in enumerate(tbs):
                        pool, ptag = pools[npool % len(pools)]
                        npool += 1
                        ps_o = pool.tile([128, 512], F32, tag=ptag,
                                         name="ps_o")
                        for k in range(4):
                            nc.tensor.matmul(
                                ps_o[:], OT_sb[:, k, tb * 128:(tb + 1) * 128],
                                wo_sb[:, k, :], start=(k == 0), stop=(k == 3))
                        o_sb = outp.tile([128, 512], F32, tag="o",
                                         name="o_sb")
                        if tb % 2 == 0:
                            nc.scalar.copy(o_sb[:], ps_o[:])
                        else:
                            nc.vector.tensor_copy(o_sb[:], ps_o[:])
                        nc.sync.dma_start(
                            y[b * S + tb * 128:b * S + (tb + 1) * 128,
                              nt * 512:(nt + 1) * 512], o_sb[:])
                        if i % 2 == 1:
                            yield
                    yield

            # ---- batch schedule ----
            # stage A tt=0,1 zipped with the previous batch's stage-C tail
            def a_front():
                for u in stage_a(0, (b, 1)):
                    yield u
                for u in stage_a(1, (b, 2)):
                    yield u

            if pending_c_tail is not None:
                _zip_chunks(a_front(), pending_c_tail)
                pending_c_tail = None
            else:
                for _ in a_front():
                    pass
            # attention qh=0 zipped with stage A tt=2,3
            def a_tail():
                for u in stage_a(2, (b, 3)):
                    yield u
                nxt = (b + 1, 0) if b + 1 < B else None
                for u in stage_a(3, nxt):
                    yield u
            _zip_chunks(a_tail(), attn_half(0))
            # attention qh=1 zipped with stage C for its ready token blocks
            _zip_chunks(stage_c([0, 1, 2, 3], [(psA, "psa")]), attn_half(1))
            # the tb>=4 tail runs zipped into the next batch's stage A,
            # drawing PSUM from the attention pools (idle in that window)
            pending_c_tail = stage_c([4, 5, 6, 7], [(psSC, "pssc"), (psOT, "psot")])

        if pending_c_tail is not None:
            for _ in pending_c_tail:
                pass

    nc.compile()
    return nc


_CACHE = {}


def _get_program(causal):
    if causal not in _CACHE:
        _CACHE[causal] = _build_program(causal)
    return _CACHE[causal]


def kernel(x, wq_w, wq_a, wq_b, wk_w, wv_w, wv_a, wv_b, wo_w,
           freqs_cos, freqs_sin, mask, start_pos=0, _trace=False):
    assert int(np.asarray(start_pos)) == 0
    shared, cores, causal = _host_prep(
        x, wq_w, wq_a, wq_b, wk_w, wv_w, wv_a, wv_b, wo_w,
        freqs_cos, freqs_sin, mask)
    nc = _get_program(causal)
    in_maps = []
    for c in range(N_CORES):
        m = dict(xT=shared["xT"], cc=shared["cc"], ss=shared["ss"],
                 maskp=shared["maskp"])
        m.update(cores[c])
        in_maps.append(m)
    res = run_bass_kernel_spmd(nc, in_maps, list(range(N_CORES)),
                               trace=_trace)
    kernel._last_results = res
    acc = np.zeros((T, D), np.float32)
    for c in range(N_CORES):
        acc += np.asarray(res.results[c]["y"], np.float32)
    out = acc.reshape(B, S, D)
    return out.astype(np.asarray(x).dtype, copy=False)


# revision 11
# speedup vs baseline: 1.6617x; 1.1246x over previous
"""Trainium2 Bass kernel for LoRA attention prefill (B=4, S=1024, D=4096, H=32).

Sharding: tensor-parallel over heads. Each of the 8 cores computes 4 heads
(512 of the 4096 q/k/v features, column-shard of wq/wk/wv) and a row-shard
of wo, producing a full-shape [T, D] partial output; partials are summed on
the host.

v3 design notes:
  - LoRA folded into wq/wv on the host (exact algebra) - no device LoRA work.
  - Causal masking is multiplicative (0/1 bf16 after exp); diagonal score
    blocks only compute the live query range (partial-width matmuls).
  - Softmax denominators: exp tiles accumulated on DVE into a bf16 SBUF acc;
    one ones-matrix matmul per head broadcasts the denominator to all 128
    partitions; reciprocal_approx_fast (DVE) replaces the slow serial
    RECIPROCAL.
  - The PE instruction stream is software-pipelined end to end: attention
    rounds (which are exp/Scalar-latency bound) are interleaved with stage-A
    projection and stage-C wo matmul chunks via generators, so the in-order
    PE queue always has independent work. PV matmuls lag two rounds behind
    their score matmuls.
  - Startup weight DMAs are split into consumption-order chunks; the next
    batch's first x tile is prefetched before stage C.
"""
import sys
from contextlib import ExitStack

sys.path.insert(0, "/opt/trn_rl_repo")

import numpy as np
import ml_dtypes

import concourse.bass as bass
import concourse.mybir as mybir
import concourse.tile as tile
from concourse import bacc
from concourse.bass_utils import run_bass_kernel_spmd
from concourse.tile import TileContext

B, S, D = 4, 1024, 4096
H, HD = 32, 128
R = 16
LORA_SCALE = 2.0
N_CORES = 8
HPC = H // N_CORES            # heads per core
FPC = HPC * HD                # features per core = 512
T = B * S                     # 4096 tokens
TT = 256                      # stage-A T-tile (tokens)
NTT = S // TT                 # T-tiles per batch = 4
SCALE = float(1.0 / np.sqrt(HD))
BF = mybir.dt.bfloat16
F32 = mybir.dt.float32


def _bf(a):
    return np.ascontiguousarray(np.asarray(a, np.float32).astype(ml_dtypes.bfloat16))


def _core_perm(c):
    hs = [HPC * c + i for i in range(HPC)]
    ev = np.arange(0, HD, 2)
    od = np.arange(1, HD, 2)
    out = []
    for pair in (0, 1):
        h0, h1 = hs[2 * pair], hs[2 * pair + 1]
        out.append(h0 * HD + ev)
        out.append(h1 * HD + ev)
        out.append(h0 * HD + od)
        out.append(h1 * HD + od)
    return np.concatenate(out)


def _check_causal(mask):
    iu = np.triu_indices(S, k=1)
    il = np.tril_indices(S, k=0)
    return bool((mask[iu] <= -1e8).all() and (mask[il] == 0).all())


def _host_prep(x, wq_w, wq_a, wq_b, wk_w, wv_w, wv_a, wv_b, wo_w,
               freqs_cos, freqs_sin, mask):
    x2 = np.asarray(x, np.float32).reshape(T, D)
    xT = _bf(x2.T)

    # fold LoRA into the dense weights: y = x(W + s*B@A)^T exactly
    wq_eff = np.asarray(wq_w, np.float32) + LORA_SCALE * (
        np.asarray(wq_b, np.float32) @ np.asarray(wq_a, np.float32))
    wv_eff = np.asarray(wv_w, np.float32) + LORA_SCALE * (
        np.asarray(wv_b, np.float32) @ np.asarray(wv_a, np.float32))
    wk = np.asarray(wk_w, np.float32)

    cosT = np.asarray(freqs_cos, np.float32).T
    sinT = np.asarray(freqs_sin, np.float32).T
    cc = np.ascontiguousarray(np.tile(cosT, (2, B)).astype(np.float32))
    ss = np.ascontiguousarray(np.tile(sinT, (2, B)).astype(np.float32))

    mask = np.asarray(mask, np.float32)
    causal = _check_causal(mask)
    if causal:
        # 0/1 multiplicative triangle for the 128x128 diagonal strips
        tri = np.tril(np.ones((128, 128), np.float32)).T  # [k,q]: 1 if k<=q
        maskp = _bf(np.broadcast_to(tri, (4, 128, 128)))
    else:
        mT = mask.T * np.float32(np.sqrt(HD))
        maskp = np.zeros((8, 128, 2, 512), np.float32)
        for qh in range(2):
            for j in range(8):
                maskp[j, :, qh, :] = mT[j * 128:(j + 1) * 128,
                                        qh * 512:(qh + 1) * 512]

    shared = dict(xT=xT, cc=cc, ss=ss, maskp=maskp)
    cores = []
    for c in range(N_CORES):
        perm = _core_perm(c)
        sl = slice(c * FPC, (c + 1) * FPC)
        cores.append(dict(
            wqT=_bf(wq_eff[perm, :].T),
            wkT=_bf(wk[perm, :].T),
            wvT=_bf(wv_eff[sl, :].T),
            woT=_bf(np.asarray(wo_w, np.float32)[:, sl].T),
        ))
    return shared, cores, causal


def _zip_chunks(*gens):
    """Round-robin drive generators to completion."""
    gens = list(gens)
    while gens:
        for g in list(gens):
            try:
                next(g)
            except StopIteration:
                gens.remove(g)


def _build_program(causal):
    nc = bacc.Bacc("TRN2", num_devices=N_CORES)

    xT = nc.dram_tensor("xT", [D, T], BF, kind="ExternalInput").ap()
    wqT = nc.dram_tensor("wqT", [D, FPC], BF, kind="ExternalInput").ap()
    wkT = nc.dram_tensor("wkT", [D, FPC], BF, kind="ExternalInput").ap()
    wvT = nc.dram_tensor("wvT", [D, FPC], BF, kind="ExternalInput").ap()
    woT = nc.dram_tensor("woT", [FPC, D], BF, kind="ExternalInput").ap()
    cc = nc.dram_tensor("cc", [128, T], F32, kind="ExternalInput").ap()
    ss = nc.dram_tensor("ss", [128, T], F32, kind="ExternalInput").ap()
    if causal:
        maskp = nc.dram_tensor("maskp", [4, 128, 128], BF,
                               kind="ExternalInput").ap()
    else:
        maskp = nc.dram_tensor("maskp", [8, 128, 2, 512], F32,
                               kind="ExternalInput").ap()
    y = nc.dram_tensor("y", [T, D], F32, kind="ExternalOutput").ap()

    with TileContext(nc) as tc, ExitStack() as ctx:
        wpool = ctx.enter_context(tc.tile_pool(name="wpool", bufs=1))
        xpool = ctx.enter_context(tc.tile_pool(name="xpool", bufs=2))
        ccp = ctx.enter_context(tc.tile_pool(name="ccp", bufs=4))
        qkvp = ctx.enter_context(tc.tile_pool(name="qkvp", bufs=1))
        expp = ctx.enter_context(tc.tile_pool(name="expp", bufs=7))
        accp = ctx.enter_context(tc.tile_pool(name="accp", bufs=4))
        recp = ctx.enter_context(tc.tile_pool(name="recp", bufs=2))
        otp = ctx.enter_context(tc.tile_pool(name="otp", bufs=1))
        outp = ctx.enter_context(tc.tile_pool(name="outp", bufs=4))
        tmpp = ctx.enter_context(tc.tile_pool(name="tmpp", bufs=4))
        stp = ctx.enter_context(tc.tile_pool(name="stp", bufs=4))
        wop = ctx.enter_context(tc.tile_pool(name="wop", bufs=2))
        psA = ctx.enter_context(tc.tile_pool(name="psA", bufs=3, space="PSUM"))
        psOT = ctx.enter_context(tc.tile_pool(name="psOT", bufs=2,
                                              space="PSUM"))
        psSC = ctx.enter_context(tc.tile_pool(name="psSC", bufs=3,
                                              space="PSUM"))

        xre = xT.rearrange("(o p) t -> p o t", p=128)
        # prefetched stage-A input tiles, keyed by (b, tt)
        fetched = {}

        def fetch_x(b, tt):
            t0 = b * S + tt * TT
            x_sb = xpool.tile([128, 32, TT], BF, tag="x", name="x_sb")
            nc.sync.dma_start(x_sb[:], xre[:, :, t0:t0 + TT])
            cc_sb = ccp.tile([128, TT], F32, tag="cc", name="cc_sb")
            nc.sync.dma_start(cc_sb[:], cc[:, t0:t0 + TT])
            ss_sb = ccp.tile([128, TT], F32, tag="ss", name="ss_sb")
            nc.sync.dma_start(ss_sb[:], ss[:, t0:t0 + TT])
            fetched[(b, tt)] = (x_sb, cc_sb, ss_sb)

        # first two input tiles before the weights so compute starts early
        fetch_x(0, 0)
        fetch_x(0, 1)

        # resident weights, split into consumption-order chunks so the first
        # projection matmuls can start before the full weight set has landed
        wq_sb = wpool.tile([128, 32, FPC], BF, tag="wq")
        wk_sb = wpool.tile([128, 32, FPC], BF, tag="wk")
        wv_sb = wpool.tile([128, 32, FPC], BF, tag="wv")
        wqr = wqT.rearrange("(o p) f -> p o f", p=128)
        wkr = wkT.rearrange("(o p) f -> p o f", p=128)
        wvr = wvT.rearrange("(o p) f -> p o f", p=128)
        for f0 in range(0, FPC, 128):
            nc.sync.dma_start(wq_sb[:, :, f0:f0 + 128], wqr[:, :, f0:f0 + 128])
        for f0 in range(0, FPC, 128):
            nc.sync.dma_start(wk_sb[:, :, f0:f0 + 128], wkr[:, :, f0:f0 + 128])
        for f0 in range(0, FPC, 256):
            nc.sync.dma_start(wv_sb[:, :, f0:f0 + 256], wvr[:, :, f0:f0 + 256])
        if causal:
            mask_sb = wpool.tile([128, 4, 128], BF, tag="mask")
            nc.sync.dma_start(mask_sb[:], maskp.rearrange("j p n -> p j n"))
        else:
            mask_sb = wpool.tile([128, 8, 2, 512], F32, tag="mask")
            nc.sync.dma_start(mask_sb[:],
                              maskp.rearrange("j p q n -> p j q n"))
        ones_m = wpool.tile([128, 128], BF, tag="onesm")
        nc.gpsimd.memset(ones_m[:], 1.0)

        pending_c_tail = None
        for b in range(B):
            Q_sb = qkvp.tile([128, 4, S], BF, tag="Qsb")
            K_sb = qkvp.tile([128, 4, S], BF, tag="Ksb")
            V_sb = qkvp.tile([128, 8, FPC], BF, tag="Vsb")
            OT_sb = otp.tile([128, 4, S], BF, tag="OT")

            def stage_a(tt, prefetch_next):
                """Generator: QK pair groups + V blocks for one t-tile."""
                x_sb, cc_sb, ss_sb = fetched.pop((b, tt))
                if prefetch_next is not None:
                    fetch_x(*prefetch_next)
                toff = tt * TT
                for dst_sb, w_sb in ((Q_sb, wq_sb), (K_sb, wk_sb)):
                    for pair in range(2):
                        ps_pair = []
                        for ri in range(2):
                            f0 = pair * 256 + ri * 128
                            ps = psA.tile([128, 512], F32, tag="psa",
                                          name="ps_qk")
                            for d in range(32):
                                nc.tensor.matmul(
                                    ps[:, 0:TT], w_sb[:, d, f0:f0 + 128],
                                    x_sb[:, d, :], start=(d == 0),
                                    stop=(d == 31))
                            ps_pair.append(ps)
                        ps_r, ps_i = ps_pair
                        t1 = tmpp.tile([128, TT], F32, tag="t", name="t1")
                        nc.vector.tensor_mul(t1[:], ps_r[:, 0:TT], cc_sb[:])
                        t2 = tmpp.tile([128, TT], F32, tag="t", name="t2")
                        nc.vector.tensor_mul(t2[:], ps_i[:, 0:TT], ss_sb[:])
                        st_r = stp.tile([128, TT], BF, tag="st", name="st_r")
                        nc.vector.tensor_tensor(
                            st_r[:], t1[:], t2[:], mybir.AluOpType.subtract)
                        t3 = tmpp.tile([128, TT], F32, tag="t", name="t3")
                        nc.vector.tensor_mul(t3[:], ps_r[:, 0:TT], ss_sb[:])
                        t4 = tmpp.tile([128, TT], F32, tag="t", name="t4")
                        nc.vector.tensor_mul(t4[:], ps_i[:, 0:TT], cc_sb[:])
                        st_i = stp.tile([128, TT], BF, tag="st", name="st_i")
                        nc.vector.tensor_tensor(
                            st_i[:], t3[:], t4[:], mybir.AluOpType.add)
                        # shuffle into head-contiguous blocks: head h of this
                        # pair = [r half; i half] on partitions [0:64|64:128]
                        for hh in range(2):
                            h_loc = 2 * pair + hh
                            nc.scalar.dma_start(
                                dst_sb[0:64, h_loc, toff:toff + TT],
                                st_r[hh * 64:(hh + 1) * 64, :])
                            nc.scalar.dma_start(
                                dst_sb[64:128, h_loc, toff:toff + TT],
                                st_i[hh * 64:(hh + 1) * 64, :])
                        yield
                for v in range(TT // 128):
                    tb = tt * (TT // 128) + v
                    ps_v = psA.tile([128, 512], F32, tag="psa", name="ps_v")
                    for d in range(32):
                        nc.tensor.matmul(
                            ps_v[:], x_sb[:, d, v * 128:(v + 1) * 128],
                            wv_sb[:, d, :], start=(d == 0), stop=(d == 31))
                    nc.scalar.copy(V_sb[:, tb, :], ps_v[:])
                    yield

            def attn_half(qh):
                """Generator: attention rounds for one query half."""
                q0 = qh * 512
                kbs = list(range(0, qh * 4 + 4)) if causal else list(range(8))
                nkb = len(kbs)
                LAG = 2

                def width(kb):
                    if causal and kb - 4 * qh >= 0:
                        return (kb - 4 * qh) * 128
                    return 0

                for hp in range(2):
                    heads = (2 * hp, 2 * hp + 1)
                    ps_ot = {}
                    acc = {}
                    e_t = {}
                    for l in heads:
                        ps_ot[l] = psOT.tile([128, 512], F32, tag="psot",
                                             name="ps_ot")
                        acc[l] = accp.tile([128, 512], BF, tag="acc",
                                           name="acc")

                    def emit_sc(l, j):
                        kb = kbs[j]
                        qlo = width(kb)
                        ps_sc = psSC.tile([128, 512], F32, tag="pssc",
                                          name="ps_sc")
                        nc.tensor.matmul(
                            ps_sc[:, qlo:512],
                            K_sb[:, l, kb * 128:kb * 128 + 128],
                            Q_sb[:, l, q0 + qlo:q0 + 512],
                            start=True, stop=True)
                        e_sb = expp.tile([128, 512], BF, tag="e", name="e_sb")
                        if causal:
                            nc.scalar.activation(
                                e_sb[:, qlo:512], ps_sc[:, qlo:512],
                                mybir.ActivationFunctionType.Exp, scale=SCALE)
                            jj = kb - 4 * qh
                            if jj >= 0:
                                nc.vector.tensor_mul(
                                    e_sb[:, qlo:qlo + 128],
                                    e_sb[:, qlo:qlo + 128],
                                    mask_sb[:, jj, :])
                        else:
                            nc.vector.tensor_add(
                                ps_sc[:], ps_sc[:], mask_sb[:, kb, qh, :])
                            nc.scalar.activation(
                                e_sb[:], ps_sc[:],
                                mybir.ActivationFunctionType.Exp, scale=SCALE)
                        e_t[(l, j)] = e_sb

                    def emit_pv(l, j):
                        kb = kbs[j]
                        qlo = width(kb)
                        e_sb = e_t.pop((l, j))
                        nc.tensor.matmul(
                            ps_ot[l][:, qlo:512],
                            V_sb[:, kb, l * 128:(l + 1) * 128],
                            e_sb[:, qlo:512],
                            start=(j == 0), stop=(j == nkb - 1))
                        if j == 0:
                            nc.vector.tensor_copy(acc[l][:], e_sb[:])
                        else:
                            nc.vector.tensor_add(
                                acc[l][:, qlo:512], acc[l][:, qlo:512],
                                e_sb[:, qlo:512])

                    # software pipeline: PV lags LAG key blocks behind scores
                    for j in range(nkb):
                        for l in heads:
                            emit_sc(l, j)
                            if j >= LAG:
                                emit_pv(l, j - LAG)
                        yield
                    for j in range(max(nkb - LAG, 0), nkb):
                        for l in heads:
                            emit_pv(l, j)
                    # normalization tail for this head pair
                    rec = {}
                    for l in heads:
                        ps_den = psSC.tile([128, 512], F32, tag="pssc",
                                           name="ps_den")
                        nc.tensor.matmul(ps_den[:], ones_m[:], acc[l][:],
                                         start=True, stop=True)
                        rec_bc = recp.tile([128, 512], F32, tag="rec",
                                           name="rec_bc")
                        nc.vector.reciprocal_approx_fast(rec_bc[:], ps_den[:])
                        rec[l] = rec_bc
                    for l in heads:
                        nc.vector.tensor_mul(OT_sb[:, l, q0:q0 + 512],
                                             ps_ot[l][:], rec[l][:])
                    yield

            def stage_c(tbs, pools, b=b, OT_sb=OT_sb):
                """Generator: wo matmuls for the given token blocks.

                pools: PSUM pools to rotate ps_o through (pass the idle
                attention pools for the cross-batch tail so drain latency is
                fully hidden)."""
                npool = 0
                for nt in range(8):
                    wo_sb = wop.tile([128, 4, 512], BF, tag="wo",
                                     name="wo_sb")
                    nc.sync.dma_start(
                        wo_sb[:],
                        woT.rearrange("(o p) n -> p o n",
                                      p=128)[:, :, nt * 512:(nt + 1) * 512])
                    for i, tb in enumerate(tbs):
                        pool, ptag = pools[npool % len(pools)]
                        npool += 1
                        ps_o = pool.tile([128, 512], F32, tag=ptag,
                                         name="ps_o")
                        for k in range(4):
                            nc.tensor.matmul(
                                ps_o[:], OT_sb[:, k, tb * 128:(tb + 1) * 128],
                                wo_sb[:, k, :], start=(k == 0), stop=(k == 3))
                        o_sb = outp.tile([128, 512], F32, tag="o",
                                         name="o_sb")
                        if tb % 2 == 0:
                            nc.scalar.copy(o_sb[:], ps_o[:])
                        else:
                            nc.vector.tensor_copy(o_sb[:], ps_o[:])
                        nc.sync.dma_start(
                            y[b * S + tb * 128:b * S + (tb + 1) * 128,
                              nt * 512:(nt + 1) * 512], o_sb[:])
                        if i % 2 == 1:
                            yield
                    yield

            # ---- batch schedule ----
            # stage A tt=0,1 zipped with the previous batch's stage-C tail
            def a_front():
                for u in stage_a(0, None if b == 0 else (b, 1)):
                    yield u
                for u in stage_a(1, (b, 2)):
                    yield u

            if pending_c_tail is not None:
                _zip_chunks(a_front(), pending_c_tail)
                pending_c_tail = None
            else:
                for _ in a_front():
                    pass
            # attention qh=0 zipped with stage A tt=2,3
            def a_tail():
                for u in stage_a(2, (b, 3)):
                    yield u
                nxt = (b + 1, 0) if b + 1 < B else None
                for u in stage_a(3, nxt):
                    yield u
            _zip_chunks(a_tail(), attn_half(0))
            # attention qh=1 zipped with stage C for its ready token blocks
            _zip_chunks(stage_c([0, 1, 2, 3], [(psA, "psa")]), attn_half(1))
            # the tb>=4 tail runs zipped into the next batch's stage A,
            # drawing PSUM from the attention pools (idle in that window)
            pending_c_tail = stage_c(
                [4, 5, 6, 7],
                [(psSC, "pssc"), (psOT, "psot")] + ([(psA, "psa")] if b == B - 1 else []))

        if pending_c_tail is not None:
            for _ in pending_c_tail:
                pass

    nc.compile()
    return nc


_CACHE = {}


def _get_program(causal):
    if causal not in _CACHE:
        _CACHE[causal] = _build_program(causal)
    return _CACHE[causal]


def kernel(x, wq_w, wq_a, wq_b, wk_w, wv_w, wv_a, wv_b, wo_w,
           freqs_cos, freqs_sin, mask, start_pos=0, _trace=False):
    assert int(np.asarray(start_pos)) == 0
    shared, cores, causal = _host_prep(
        x, wq_w, wq_a, wq_b, wk_w, wv_w, wv_a, wv_b, wo_w,
        freqs_cos, freqs_sin, mask)
    nc = _get_program(causal)
    in_maps = []
    for c in range(N_CORES):
        m = dict(xT=shared["xT"], cc=shared["cc"], ss=shared["ss"],
                 maskp=shared["maskp"])
        m.update(cores[c])
        in_maps.append(m)
    res = run_bass_kernel_spmd(nc, in_maps, list(range(N_CORES)),
                               trace=_trace)
    kernel._last_results = res
    acc = np.zeros((T, D), np.float32)
    for c in range(N_CORES):
        acc += np.asarray(res.results[c]["y"], np.float32)
    out = acc.reshape(B, S, D)
    return out.astype(np.asarray(x).dtype, copy=False)
